# revision 12
# baseline (speedup 1.0000x reference)
"""Trainium2 Bass kernel: inclusive cumsum along L for X (4, 8192, 32, 32) f32.

Hybrid quad-PE + custom-DVE design (8 NeuronCores, SPMD), bf16 HBM traffic:
  - Shard: core i gets b = i//2, channel-half = i%2 -> 512 channels x 8192 L,
    split 128 (PE sideband) / 384 (DVE pipeline). HBM per core: 8 MiB in +
    8 MiB out at ~420 GB/s aggregate over 3 DMA rings (~40 us).
  - DVE (384 ch): host-transposed [3][128ch][8192L] bf16. Custom DVE op
    ANT_CUMSUM_INIT (registered per-NEFF: body = scan(ADD, Src0, init=C0))
    scans the free dim at 1 elem/cycle (~1.06 ns/elem, 2x the stock
    tensor_tensor_scan) with fp32 ALU state; 12 chunk-scans of [128, 2048]
    chained through an f32 [P,1] state column (imm0 scalars must be f32).
  - PE (128 ch): Blelloch scan restructured for INSTRUCTION COUNT — at
    <=512 output cols the tensor pipeline is overhead-bound (~330 ns per
    matmul), so the per-block formulation (158 matmuls, ~52 us) loses to
    this QUAD formulation (~68 matmuls, 512 cols each):
      phase 1 (4 mm/group): one-hot [128,4] stationaries accumulate quad
        column-sum rows S4[q, 512] = [S_4q|S_4q+1|S_4q+2|S_4q+3];
      carry (gpsimd + 5 mm/group): gpsimd builds QS = sum of quarters,
        QS4 = QS replicated, TI = intra-quad prefixes [S0|S0+S1|S0+S1+S2];
        PE computes T4 = ones⊗ca4 + tms4_strict@QS4 + I4@TI (quarters
        1..3), whose row q quarter j = T_{block 4q+j}; next carry via
        ones_4x1@QS + ca.
      phase 3 (8 mm/group): per quad ONE row-selector matmul broadcasts
        tb4 row q onto [128, 512] (all 4 quarters get their block's T) and
        ONE 512-col upper-triangular matmul accumulates the within-block
        prefixes. ScalarE drains each bank (16 copies of [128, 512]).
  - DMA rings: sync = PE ins + DVE tile-0 ins + tile-0 outs (6.3 MiB);
    scalar = DVE tile-1/2 ins + PE outs (6.3 MiB); gpsimd SWDGE = DVE
    tile-1/2 outs (4.2 MiB). Engine streams are kept head-of-line-safe:
    the PE chain is emitted BEFORE the DVE chain so gpsimd runs
    consts -> carry micro-ops (PE-paced) -> yv outs (scan-paced).
  - Error budget (tolerance 2e-2 * max|out| ~ 9.1): bf16 input quant ~0.3,
    bf16 carry/chunk chains ~2.7 each worst-case, output rounding ~1.8.
"""

import numpy as np
import ml_dtypes
from contextlib import ExitStack

import concourse.bass as bass
import concourse.tile as tile
from concourse import bacc, masks, mybir
from concourse.bass_utils import run_bass_kernel_spmd

N_CORES = 8
B, L, D, N = 4, 8192, 32, 32
C_FULL = D * N          # 1024 channels total
CH = C_FULL // 2        # 512 channels per core
C = 128                 # PE-half channels
CV = CH - C             # DVE-half channels (384)
P = 128
NBLK = L // P           # 64 blocks of 128 rows
QBLK = 4                # blocks per quad (one 512-col PSUM bank)
GBLK = 16               # blocks per carry group (4 quads)
NGRP = NBLK // GBLK     # 4 groups
SBB = 8                 # blocks per superblock (DMA unit)
NSB = NBLK // SBB       # 8 superblocks
SBW = SBB * C           # 1024 cols per superblock tile
QW = QBLK * C           # 512 cols per quad
NVT = CV // P           # 3 DVE tiles of 128 channels
VCH = 2048              # DVE chunk width (cols of L)
NVC = L // VCH          # 4 chunks per tile

_CACHE = {}


def _register_cumsum_op():
    """Per-NEFF custom DVE op: out[p,k] = s0[p] + sum_{j<=k} in0[p,j].
    Appended to dve_ops.OPS with a computed uops_sha (the documented
    per-NEFF DVE-table extension point); runs at 1 elem/cycle."""
    from concourse import dve_ops
    from concourse.dve_spec import Spec, Src0, C0, AluOp, scan, lower
    from concourse.dve_uop import DveOpSpec

    name = "ANT_CUMSUM_INIT"
    for op in dve_ops.OPS:
        if op.name == name:
            return op
    spec = Spec(
        body=scan(AluOp.ADD, Src0, init=C0),
        reference=lambda in0, s0: np.cumsum(in0.astype(np.float32), axis=-1)
        + np.asarray(s0, dtype=np.float32),
    )
    row = dve_ops._CUSTOM_DVE_ROW_BASE + len(dve_ops.OPS)
    sha = {}
    for ver in ("v3", "v4"):
        s = DveOpSpec(name=name, opcode=row, uops=lower(spec, ver=ver), rd1_en=False)
        sha[ver] = s.sha(ver)
    op = dve_ops.DveOp(name, spec, subdim=False, uops_sha=sha)
    dve_ops.OPS.append(op)
    dve_ops._SUB_OPCODE_FOR_NAME[name] = row
    dve_ops.CUSTOM_DVE_SPECS[name] = spec
    return op


def _build_program():
    f32 = mybir.dt.float32
    bf16 = mybir.dt.bfloat16
    add = mybir.AluOpType.add
    cumsum_op = _register_cumsum_op()
    nc = bacc.Bacc(
        trn_type="TRN2", debug=False, num_devices=N_CORES, num_swdge_queues=2
    )
    xp = nc.dram_tensor("xp", [NSB, P, SBW], bf16, kind="ExternalInput").ap()
    xv = nc.dram_tensor("xv", [NVT, P, L], bf16, kind="ExternalInput").ap()
    yp = nc.dram_tensor("yp", [NSB, P, SBW], bf16, kind="ExternalOutput").ap()
    yv = nc.dram_tensor("yv", [NVT, P, L], bf16, kind="ExternalOutput").ap()

    with tile.TileContext(nc) as tc, ExitStack() as ctx:
        const_pool = ctx.enter_context(tc.tile_pool(name="const", bufs=1))
        xin_pool = ctx.enter_context(tc.tile_pool(name="xin", bufs=1))
        xv_pool = ctx.enter_context(tc.tile_pool(name="xv", bufs=1))
        yv_pool = ctx.enter_context(tc.tile_pool(name="yv", bufs=1))
        yout_pool = ctx.enter_context(tc.tile_pool(name="yout", bufs=4))
        small_pool = ctx.enter_context(tc.tile_pool(name="small", bufs=2))
        yps_pool = ctx.enter_context(tc.tile_pool(name="yps", bufs=5, space="PSUM"))
        sps_pool = ctx.enter_context(tc.tile_pool(name="sps", bufs=1, space="PSUM"))
        tps_pool = ctx.enter_context(tc.tile_pool(name="tps", bufs=1, space="PSUM"))

        # ---- constants (gpsimd; run while the in-DMAs fly) ----
        ut = const_pool.tile([P, P], bf16, name="ut")
        masks.make_upper_triangular(nc, ut[:], 1.0, diag=True)
        # zq: ones in column QBLK-1 of a [P, 2*QBLK-1] strip; slice
        # [:, QBLK-1-q : 2*QBLK-1-q] puts the ones-column at position q.
        zq = const_pool.tile([P, 2 * QBLK - 1], bf16, name="zq")
        nc.gpsimd.memset(zq[:], 0.0)
        nc.gpsimd.memset(zq[:, QBLK - 1 : QBLK], 1.0)
        # rz4: [4, 4*128] row-selector bank; slice [:, q*128:(q+1)*128] is
        # all-ones in row q.
        rz4 = const_pool.tile([QBLK, QBLK * P], bf16, name="rz4")
        nc.gpsimd.memset(rz4[:], 1.0)
        nc.gpsimd.affine_select(
            out=rz4[:], in_=rz4[:], compare_op=mybir.AluOpType.is_ge,
            fill=0.0, base=0, pattern=[[1, QBLK * P]], channel_multiplier=-P,
        )
        nc.gpsimd.affine_select(
            out=rz4[:], in_=rz4[:], compare_op=mybir.AluOpType.is_ge,
            fill=0.0, base=P - 1, pattern=[[-1, QBLK * P]], channel_multiplier=P,
        )
        tms4 = const_pool.tile([QBLK, QBLK], bf16, name="tms4")
        masks.make_upper_triangular(nc, tms4[:], 1.0, diag=False)
        # i4: 4x4 identity (band of width 1 via two affine_selects)
        i4 = const_pool.tile([QBLK, QBLK], bf16, name="i4")
        nc.gpsimd.memset(i4[:], 1.0)
        nc.gpsimd.affine_select(
            out=i4[:], in_=i4[:], compare_op=mybir.AluOpType.is_ge,
            fill=0.0, base=0, pattern=[[1, QBLK]], channel_multiplier=-1,
        )
        nc.gpsimd.affine_select(
            out=i4[:], in_=i4[:], compare_op=mybir.AluOpType.is_ge,
            fill=0.0, base=0, pattern=[[-1, QBLK]], channel_multiplier=1,
        )
        ones_1x4 = const_pool.tile([1, QBLK], bf16, name="ones_1x4")
        nc.gpsimd.memset(ones_1x4[:], 1.0)
        ones_4x1 = const_pool.tile([QBLK, 1], bf16, name="ones_4x1")
        nc.gpsimd.memset(ones_4x1[:], 1.0)
        one_1x1 = const_pool.tile([1, 1], bf16, name="one_1x1")
        nc.gpsimd.memset(one_1x1[:], 1.0)
        ca0 = const_pool.tile([1, C], bf16, name="ca0")
        nc.gpsimd.memset(ca0[:], 0.0)
        ca0d = const_pool.tile([1, QW], bf16, name="ca0d")
        nc.gpsimd.memset(ca0d[:], 0.0)

        # ---- all in-DMAs up front, in consumption order per ring ----
        # sync ring: PE superblocks + DVE tile-0; scalar ring: DVE tile-1/2.
        xts = {}
        xvc = {}

        def in_xp(s):
            xt = xin_pool.tile([P, SBW], bf16, name=f"xt{s}", tag=f"xt{s}", bufs=1)
            nc.sync.dma_start(out=xt[:], in_=xp[s])
            xts[s] = xt

        def in_xv(t, c, eng):
            xc = xv_pool.tile(
                [P, VCH], bf16, name=f"xv{t}_{c}", tag=f"xv{t}_{c}", bufs=1
            )
            eng.dma_start(out=xc[:], in_=xv[t, :, c * VCH : (c + 1) * VCH])
            xvc[(t, c)] = xc

        in_xp(0)
        in_xv(0, 0, nc.sync)
        in_xp(1)
        in_xv(0, 1, nc.sync)
        in_xp(2)
        in_xv(0, 2, nc.sync)
        in_xp(3)
        in_xv(0, 3, nc.sync)
        for s in range(4, NSB):
            in_xp(s)
        for t in range(1, NVT):
            for c in range(NVC):
                in_xv(t, c, nc.scalar)

        def xsl(blk, nblk):
            s, k = blk // SBB, blk % SBB
            return xts[s][:, k * C : (k + nblk) * C]

        # ---- PE half: quad-formulated Blelloch scan ----
        prev_ca = ca0      # running carry [1, C]
        prev_ca4 = ca0d    # running carry duplicated x4 [1, QW]

        def emit_phase1(g):
            # S4[q, j*C:(j+1)*C] = column sums of block 4q+j (one matmul
            # per quad; one-hot stationary col q writes row q, others 0)
            sp4 = sps_pool.tile([QBLK, QW], f32, name="sp4", tag="sp4", bufs=1)
            for q in range(QBLK):
                nc.tensor.matmul(
                    sp4[:],
                    zq[:, QBLK - 1 - q : 2 * QBLK - 1 - q],
                    xsl(GBLK * g + QBLK * q, QBLK),
                    start=(q == 0),
                    stop=(q == QBLK - 1),
                )
            sa4 = small_pool.tile([QBLK, QW], bf16, name="sa4", tag="sa4", bufs=2)
            nc.scalar.copy(sa4[:], sp4[:])
            return sa4

        def emit_carry_math(g, sa4):
            nonlocal prev_ca, prev_ca4
            ca, ca4 = prev_ca, prev_ca4
            # gpsimd: QS = sum of the 4 quarters; QS4 = QS replicated x4;
            # TI = [S0 | S0+S1 | S0+S1+S2] (intra-quad exclusive prefixes
            # for quarters 1..3)
            qs = small_pool.tile([QBLK, C], bf16, name="qs", tag="qs", bufs=2)
            nc.gpsimd.tensor_tensor(
                out=qs[:], in0=sa4[:, 0:C], in1=sa4[:, C : 2 * C], op=add
            )
            nc.gpsimd.tensor_tensor(
                out=qs[:], in0=qs[:], in1=sa4[:, 2 * C : 3 * C], op=add
            )
            nc.gpsimd.tensor_tensor(
                out=qs[:], in0=qs[:], in1=sa4[:, 3 * C : 4 * C], op=add
            )
            qs4 = small_pool.tile([QBLK, QW], bf16, name="qs4", tag="qs4", bufs=2)
            for j in range(QBLK):
                nc.gpsimd.tensor_copy(qs4[:, j * C : (j + 1) * C], qs[:])
            ti = small_pool.tile([QBLK, 3 * C], bf16, name="ti", tag="ti", bufs=2)
            nc.gpsimd.tensor_copy(ti[:, 0:C], sa4[:, 0:C])
            nc.gpsimd.tensor_tensor(
                out=ti[:, C : 2 * C], in0=ti[:, 0:C], in1=sa4[:, C : 2 * C], op=add
            )
            nc.gpsimd.tensor_tensor(
                out=ti[:, 2 * C : 3 * C], in0=ti[:, C : 2 * C],
                in1=sa4[:, 2 * C : 3 * C], op=add,
            )
            # PE: T4 = ones_1x4 (x) ca4  +  tms4_strict @ QS4  +  I4 @ TI
            tp4 = tps_pool.tile([QBLK, QW], f32, name="tp4", tag="tp4", bufs=1)
            nc.tensor.matmul(tp4[:], ones_1x4[:], ca4[:], start=True, stop=False)
            nc.tensor.matmul(tp4[:], tms4[:], qs4[:], start=False, stop=False,
                             skip_group_check=True)
            nc.tensor.matmul(tp4[:, C:QW], i4[:], ti[:], start=False, stop=True,
                             skip_group_check=True)
            tb4 = small_pool.tile([QBLK, QW], bf16, name="tb4", tag="tb4", bufs=2)
            nc.scalar.copy(tb4[:], tp4[:])
            if g < NGRP - 1:
                cp = tps_pool.tile([1, C], f32, name="cp", tag="cp", bufs=1)
                nc.tensor.matmul(cp[:], ones_4x1[:], qs[:], start=True, stop=False)
                nc.tensor.matmul(cp[:], one_1x1[:], ca[:], start=False, stop=True)
                nca = small_pool.tile([1, C], bf16, name="nca", tag="nca", bufs=2)
                nc.scalar.copy(nca[:], cp[:])
                nca4 = small_pool.tile([1, QW], bf16, name="nca4", tag="nca4", bufs=2)
                for j in range(QBLK):
                    nc.scalar.copy(nca4[:, j * C : (j + 1) * C], cp[:])
                prev_ca, prev_ca4 = nca, nca4
            return tb4

        def emit_phase3(g, tb4):
            # per quad: ONE rz4 broadcast (row q of tb4 -> [128, 512], each
            # quarter gets its block's T) + ONE 512-col UT matmul. Two
            # quads per cluster share the UT stationary load.
            yt = {}
            for q0 in (0, 2):
                pend = []
                for q in (q0, q0 + 1):
                    blk = GBLK * g + QBLK * q
                    s, k = blk // SBB, blk % SBB
                    if k == 0:
                        yt[s] = yout_pool.tile(
                            [P, SBW], bf16, name=f"yt{s}", tag="yt", bufs=4
                        )
                    ps = yps_pool.tile([P, QW], f32, name="ypp", tag="ypp", bufs=5)
                    nc.tensor.matmul(
                        ps[:], rz4[:, q * P : (q + 1) * P], tb4[:],
                        start=True, stop=False,
                    )
                    pend.append((blk, ps))
                for blk, ps in pend:
                    s, k = blk // SBB, blk % SBB
                    nc.tensor.matmul(
                        ps[:], ut[:], xsl(blk, QBLK),
                        start=False, stop=True, skip_group_check=True,
                    )
                    nc.scalar.copy(yt[s][:, k * C : (k + QBLK) * C], ps[:])
                    if k + QBLK == SBB:
                        nc.scalar.dma_start(out=yp[s], in_=yt[s][:])

        # schedule: ph_0, ph_1, T_0, p3_0, ph_2, T_1, p3_1, ph_3, T_2,
        # p3_2, T_3, p3_3 (phase1 runs ahead so carries are ready early)
        sas = {}
        tbs = {}
        sas[0] = emit_phase1(0)
        sas[1] = emit_phase1(1)
        tbs[0] = emit_carry_math(0, sas[0])
        emit_phase3(0, tbs[0])
        sas[2] = emit_phase1(2)
        tbs[1] = emit_carry_math(1, sas[1])
        emit_phase3(1, tbs[1])
        sas[3] = emit_phase1(3)
        tbs[2] = emit_carry_math(2, sas[2])
        emit_phase3(2, tbs[2])
        tbs[3] = emit_carry_math(3, sas[3])
        emit_phase3(3, tbs[3])

        # ---- DVE half: chunked custom scans, chained via f32 state col ----
        # (emitted AFTER the PE chain so the gpsimd stream is consts ->
        # carry micro-ops -> yv outs, with no head-of-line blocking)
        for t in range(NVT):
            st = yv_pool.tile([P, NVC], f32, name=f"st{t}", tag=f"st{t}", bufs=1)
            for c in range(NVC):
                yc = yv_pool.tile(
                    [P, VCH], bf16, name=f"yv{t}_{c}", tag=f"yv{t}_{c}", bufs=1
                )
                init = 0.0 if c == 0 else st[:, c - 1 : c]
                nc.vector._custom_dve(
                    cumsum_op, out=yc[:], in0=xvc[(t, c)][:], s0=init
                )
                if c < NVC - 1:
                    nc.vector.tensor_copy(st[:, c : c + 1], yc[:, VCH - 1 : VCH])
                (nc.sync if t == 0 else nc.gpsimd).dma_start(
                    out=yv[t, :, c * VCH : (c + 1) * VCH], in_=yc[:]
                )

    nc.compile()
    return nc


def _get_program():
    if "nc" not in _CACHE:
        _CACHE["nc"] = _build_program()
    return _CACHE["nc"]


def _shard(X):
    Xv = X.reshape(B, L, C_FULL)
    shards = []
    for i in range(N_CORES):
        b, h = i // 2, i % 2
        slab = Xv[b, :, h * CH : (h + 1) * CH]          # [L, 512] f32
        pe = slab[:, :C]                                 # [L, 128]
        dv = slab[:, C:]                                 # [L, 384]
        arr_p = (
            pe.reshape(NSB, SBB, P, C).transpose(0, 2, 1, 3).reshape(NSB, P, SBW)
        )
        arr_v = np.ascontiguousarray(dv.T).reshape(NVT, P, L)
        shards.append(
            {
                "xp": np.ascontiguousarray(arr_p).astype(ml_dtypes.bfloat16),
                "xv": arr_v.astype(ml_dtypes.bfloat16),
            }
        )
    return shards


def _unshard(parts):
    out = np.empty((B, L, C_FULL), dtype=np.float32)
    for i in range(N_CORES):
        b, h = i // 2, i % 2
        arr_p = np.asarray(parts[i]["yp"]).astype(np.float32)
        slab_p = (
            arr_p.reshape(NSB, P, SBB, C).transpose(0, 2, 1, 3).reshape(L, C)
        )
        out[b, :, h * CH : h * CH + C] = slab_p
        arr_v = np.asarray(parts[i]["yv"]).astype(np.float32)
        out[b, :, h * CH + C : (h + 1) * CH] = arr_v.reshape(CV, L).T
    return out.reshape(B, L, D, N)


def kernel(X_in, _trace=False, _tmpdir=None, _trace_cores=None):
    X = np.asarray(X_in, dtype=np.float32)
    assert X.shape == (B, L, D, N), X.shape
    nc = _get_program()
    in_maps = _shard(X)
    kwargs = {}
    if _trace:
        kwargs = dict(
            trace=True,
            tmpdir=_tmpdir,
            trace_cores=_trace_cores or list(range(N_CORES)),
        )
    res = run_bass_kernel_spmd(nc, in_maps, core_ids=list(range(N_CORES)), **kwargs)
    out = _unshard(
        [{"yp": res.results[i]["yp"], "yv": res.results[i]["yv"]} for i in range(N_CORES)]
    )
    kernel.last_results = res
    return out


# revision 13
# speedup vs baseline: 1.0919x; 1.0919x over previous
"""Trainium2 Bass kernel: inclusive cumsum along L for X (4, 8192, 32, 32) f32.

Full-DVE design (8 NeuronCores, SPMD), bf16 HBM traffic both ways:
  - Shard: core i gets b = i//2, channel-half = i%2 -> 512 channels x 8192 L,
    host-transposed to [4][128ch][8192L] bf16 (channels on partitions, L on
    the free dim). HBM traffic per core: 8 MiB in + 8 MiB out.
  - Scan: a custom DVE op (ANT_CUMSUM_INIT, registered per-NEFF through the
    documented dve_ops extension point: body = scan(ADD, Src0, init=C0))
    computes the inclusive prefix along the free dim at 1 elem/cycle
    (~1.06 ns/elem measured — 2x the stock tensor_tensor_scan, whose
    feedback-bubble uOp costs 2 cycles/elem) with fp32 ALU state. 16
    chunk-scans of [128, 2048] per core (~2.35 us each, ~38 us total);
    chunks chain through an f32 [P,1] state column (imm0 scalar APs must
    be f32; bf16 state at 3 boundaries/row keeps the error well under
    tolerance). The PE-based Blelloch variant was abandoned: at <=256
    output columns the tensor pipeline is instruction-overhead-bound
    (~330 ns/matmul regardless of width), so its ~160-instruction scan
    never beats the DVE path, and the HAM clock governor adds variance.
  - DMA rings (each HWDGE ring sustains ~180-210 GB/s; ~420 GB/s
    aggregate): sync ring carries tile-0/1 ins + tile-0 outs (6.3 MiB);
    scalar ring carries tile-2/3 ins + tile-1 outs (6.3 MiB); gpsimd
    SWDGE ring carries tile-2/3 outs (4.2 MiB). All ins are issued first,
    in DVE consumption order, so the scan pipeline is DMA-fed ~2.4
    us/chunk against ~2.35 us/chunk consumption; no ring carries more
    than 6.3 MiB (putting 8.4 MiB on one ring starved the scans for ~8 us
    mid-kernel).
  - Error budget (tolerance 2e-2 * max|out| ~ 9.1): bf16 input quantization
    random-walks to ~0.3; bf16 chunk chaining ~2.7 worst-case; bf16 output
    rounding ~1.8. Measured ~2.4 abs (5e-3 relative).
"""

import numpy as np
import ml_dtypes
from contextlib import ExitStack

import concourse.bass as bass
import concourse.tile as tile
from concourse import bacc, mybir
from concourse.bass_utils import run_bass_kernel_spmd

N_CORES = 8
B, L, D, N = 4, 8192, 32, 32
C_FULL = D * N          # 1024 channels total
CH = C_FULL // 2        # 512 channels per core
P = 128
NVT = CH // P           # 4 DVE tiles of 128 channels
VCH = 2048              # chunk width (cols of L)
NVC = L // VCH          # 4 chunks per tile

_CACHE = {}


def _register_cumsum_op():
    """Per-NEFF custom DVE op: out[p,k] = s0[p] + sum_{j<=k} in0[p,j].
    Appended to dve_ops.OPS with a computed uops_sha (the documented
    per-NEFF DVE-table extension point); runs at 1 elem/cycle."""
    from concourse import dve_ops
    from concourse.dve_spec import Spec, Src0, C0, AluOp, scan, lower
    from concourse.dve_uop import DveOpSpec

    name = "ANT_CUMSUM_INIT"
    for op in dve_ops.OPS:
        if op.name == name:
            return op
    spec = Spec(
        body=scan(AluOp.ADD, Src0, init=C0),
        reference=lambda in0, s0: np.cumsum(in0.astype(np.float32), axis=-1)
        + np.asarray(s0, dtype=np.float32),
    )
    row = dve_ops._CUSTOM_DVE_ROW_BASE + len(dve_ops.OPS)
    sha = {}
    for ver in ("v3", "v4"):
        s = DveOpSpec(name=name, opcode=row, uops=lower(spec, ver=ver), rd1_en=False)
        sha[ver] = s.sha(ver)
    op = dve_ops.DveOp(name, spec, subdim=False, uops_sha=sha)
    dve_ops.OPS.append(op)
    dve_ops._SUB_OPCODE_FOR_NAME[name] = row
    dve_ops.CUSTOM_DVE_SPECS[name] = spec
    return op


def _build_program():
    f32 = mybir.dt.float32
    bf16 = mybir.dt.bfloat16
    cumsum_op = _register_cumsum_op()
    nc = bacc.Bacc(
        trn_type="TRN2", debug=False, num_devices=N_CORES, num_swdge_queues=2
    )
    xv = nc.dram_tensor("xv", [NVT, P, L], bf16, kind="ExternalInput").ap()
    yv = nc.dram_tensor("yv", [NVT, P, L], bf16, kind="ExternalOutput").ap()

    with tile.TileContext(nc) as tc, ExitStack() as ctx:
        xv_pool = ctx.enter_context(tc.tile_pool(name="xv", bufs=1))
        yv_pool = ctx.enter_context(tc.tile_pool(name="yv", bufs=1))

        # ---- all in-DMAs up front, in DVE consumption order per ring ----
        # sync ring: tiles 0-1; scalar ring: tiles 2-3 (land early, consumed
        # late). Interleave emission so both rings start immediately.
        xvc = {}

        def in_xv(t, c, eng):
            xc = xv_pool.tile(
                [P, VCH], bf16, name=f"xv{t}_{c}", tag=f"xv{t}_{c}", bufs=1
            )
            eng.dma_start(out=xc[:], in_=xv[t, :, c * VCH : (c + 1) * VCH])
            xvc[(t, c)] = xc

        for c in range(NVC):
            in_xv(0, c, nc.sync)
            in_xv(2, c, nc.scalar)
        for c in range(NVC):
            in_xv(1, c, nc.sync)
            in_xv(3, c, nc.scalar)

        # ---- chunked custom scans, chained via an f32 state column ----
        for t in range(NVT):
            st = yv_pool.tile([P, NVC], f32, name=f"st{t}", tag=f"st{t}", bufs=1)
            for c in range(NVC):
                yc = yv_pool.tile(
                    [P, VCH], bf16, name=f"yv{t}_{c}", tag=f"yv{t}_{c}", bufs=1
                )
                init = 0.0 if c == 0 else st[:, c - 1 : c]
                nc.vector._custom_dve(
                    cumsum_op, out=yc[:], in0=xvc[(t, c)][:], s0=init
                )
                if c < NVC - 1:
                    nc.vector.tensor_copy(st[:, c : c + 1], yc[:, VCH - 1 : VCH])
                oeng = nc.sync if t == 0 else (nc.scalar if t == 1 else nc.gpsimd)
                oeng.dma_start(
                    out=yv[t, :, c * VCH : (c + 1) * VCH], in_=yc[:]
                )

    nc.compile()
    return nc


def _get_program():
    if "nc" not in _CACHE:
        _CACHE["nc"] = _build_program()
    return _CACHE["nc"]


def _shard(X):
    Xv = X.reshape(B, L, C_FULL)
    shards = []
    for i in range(N_CORES):
        b, h = i // 2, i % 2
        slab = Xv[b, :, h * CH : (h + 1) * CH]          # [L, 512] f32
        arr_v = np.ascontiguousarray(slab.T).reshape(NVT, P, L)
        shards.append({"xv": arr_v.astype(ml_dtypes.bfloat16)})
    return shards


def _unshard(parts):
    out = np.empty((B, L, C_FULL), dtype=np.float32)
    for i in range(N_CORES):
        b, h = i // 2, i % 2
        arr_v = np.asarray(parts[i]).astype(np.float32)
        out[b, :, h * CH : (h + 1) * CH] = arr_v.reshape(CH, L).T
    return out.reshape(B, L, D, N)


def kernel(X_in, _trace=False, _tmpdir=None, _trace_cores=None):
    X = np.asarray(X_in, dtype=np.float32)
    assert X.shape == (B, L, D, N), X.shape
    nc = _get_program()
    in_maps = _shard(X)
    kwargs = {}
    if _trace:
        kwargs = dict(
            trace=True,
            tmpdir=_tmpdir,
            trace_cores=_trace_cores or list(range(N_CORES)),
        )
    res = run_bass_kernel_spmd(nc, in_maps, core_ids=list(range(N_CORES)), **kwargs)
    out = _unshard([res.results[i]["yv"] for i in range(N_CORES)])
    kernel.last_results = res
    return out
